# revision 39
# baseline (speedup 1.0000x reference)
"""Trainium2 Bass kernel: 3-level threshold activation (elementwise).

  x <  0.33          -> f32(0.333333333)  (= f32 1/3)
  0.33 <= x < 0.66   -> f32(0.6666666666) (= f32 2/3)
  x >= 0.66          -> 1.0

The output has only 3 distinct values, so the device packs FOUR 2-bit
codes per byte and the host decodes them with shifts + a LUT -
bit-identical to the jnp reference. HBM traffic per core is ~34 MB
(32 read + 2 write); one sync-ring HWDGE queue sustains ~404 GB/s of
loads with zero gaps, so the load stream (~83 us) is the floor.

Per-element planes (same math as the proven baseline):
  DVE:     plane0 = (x is_ge 0.33)                 fp8 {0,1}, full width
  ScalarE: plane1 = Sign(-2^20 x + 2^20*f32(0.66)) fp8 {+1,-1}, sign cols
           (exact: 2^20*x and the diff are exact in f32; x never equals
            f32(0.66) on the 2^-23 input grid, so Sign never sees 0)
  DVE:     plane1 = (x is_ge 0.66) on the rest     fp8 {0,1}
  PE:      psum[i,n] = sum_p W[p,i] * (plane0+plane1)[p,n] via two
           accumulated fp8 matmuls per 512-col chunk; static weights
           W[p,i] = 4^(p-4i).  -> code {1,2,0} (sign cols) / {0,1,2}
           (natural cols); PSUM sums <= 170, exact in f32.
           [96|64, 1024] PSUM mega-tiles batch 3/3/2 row-blocks (matmul
           dst base partition is limited to 0/32/64); the three col
           strips stream concurrently on separate XBUSes when warm.
  casts:   PSUM -> u8 into [96|96|64, width] staging tiles, stored with
           >= 1 KB contiguous lines.

Column blocks are 4096/2048/2048 wide: the NARROW final block halves
everything on the post-last-load critical path (planes, the FIFO-gated
last psum group's matmuls, casts, stores).

Engine dedication (the original baseline starved its load rings when a
compute op stalled, because loads were issued from the scalar engine
between ACTIVATEs, and stores head-of-line-blocked loads on sync):
  Sync ring:   ALL 24 input loads, issued back-to-back
  GpSimd ring: weight load + output stores (SWDGE, own queue, overlaps
               the load stream - serializing stores costs far more)
  ScalarE:     Sign + ~15 casts, no DMA issue
  DVE:         is_ge planes + ~9 casts, no DMA issue
PE matmuls are emitted group-major per block: the PE drains its strict
FIFO of everything ready before the last group's load-gated matmuls.
Sharding: 8192 rows split evenly across 8 NeuronCores, data parallel.
"""

import numpy as np

import concourse.bacc as bacc
import concourse.tile as tile
from concourse import mybir
from concourse.bass_utils import run_bass_kernel_spmd

N_CORES = 8
ROWS, COLS = 8192, 8192
SHARD_ROWS = ROWS // N_CORES  # 1024
P = 128
HALF = 1024       # psum mega-tile width (2 PSUM banks)
CH = 512          # matmul moving-dim chunk
PACK = 4
OP = P // PACK    # 32 packed rows per row-block
RB = SHARD_ROWS // P   # 8 row-blocks

T1 = 0.33
T2 = 0.66
ACT_SCALE = -float(2.0 ** 20)
ACT_BIAS = float(np.float32(T2) * np.float32(2.0 ** 20))  # 692060.1875
LUT6 = np.array([1.0, 0.333333333, 0.6666666666,
                 0.333333333, 0.6666666666, 1.0], dtype=np.float32)

# column blocks: (start col, width, sign-coverage cols). Sign covers
# ~65% of each block (ScalarE), is_ge the rest (DVE) - engine balance.
BLOCKS = ((0, 4096, 2816), (4096, 2048, 1280), (6144, 2048, 1280))

# psum groups: (group idx, row-blocks, packed-row base)
GROUPS = ((0, (0, 1, 2), 0), (1, (3, 4, 5), 96), (2, (6, 7), 192))
# cast engine assignment per (g*halves+h), by block width. Tail blocks
# put g2's last cast on ACT and the other on DVE so the two tail casts
# run in parallel on both engines.
_ACT_CASTS = {4096: {0, 1, 2, 4, 5, 10, 11}, 2048: {0, 1, 3, 5}}

_BUILT = {}


def _weights() -> np.ndarray:
    w = np.zeros((P, OP), dtype=np.float32)
    for p in range(P):
        w[p, p // PACK] = float(4 ** (p % PACK))
    return w.astype(mybir.dt.np(mybir.dt.float8e4))


def build_nc(shard_rows: int = SHARD_ROWS, cols: int = COLS):
    nc = bacc.Bacc(
        "TRN2",
        target_bir_lowering=False,
        debug=False,
        num_devices=N_CORES,
    )
    x = nc.dram_tensor("inputs", [shard_rows, cols], mybir.dt.float32,
                       kind="ExternalInput").ap()
    w = nc.dram_tensor("w", [P, OP], mybir.dt.float8e4,
                       kind="ExternalInput").ap()
    bias_d = nc.dram_tensor("bias", [P, 1], mybir.dt.float32,
                            kind="ExternalInput").ap()
    o = nc.dram_tensor("out", [shard_rows // PACK, cols], mybir.dt.uint8,
                       kind="ExternalOutput").ap()

    fp8 = mybir.dt.float8e4
    f32 = mybir.dt.float32
    n_blk = len(BLOCKS)
    with tile.TileContext(nc) as tc:
        with tc.tile_pool(name="wp", bufs=1) as wp, \
             tc.tile_pool(name="xp", bufs=6) as xp, \
             tc.tile_pool(name="cbp", bufs=9) as cbp, \
             tc.tile_pool(name="stp", bufs=4) as stp, \
             tc.psum_pool(name="psp", bufs=4) as psp:
            wt = wp.tile([P, OP], fp8)
            nc.gpsimd.dma_start(out=wt[:], in_=w[:, :])
            # Sign bias const arrives by DMA through a pool-tracked tile:
            # Tile's dependency on the first Sign replaces the (slower)
            # memset + all-engine-barrier preamble of earlier versions.
            bt = wp.tile([P, 1], f32, name="bt")
            nc.gpsimd.dma_start(out=bt[:], in_=bias_d[:, :])
            nc.const_aps.aps[(mybir.dt.float32, ACT_BIAS)] = bt

            def flush(job):
                ps, st, h, on_act, stores = job
                dst = st[:, h * HALF:(h + 1) * HALF]
                if on_act:
                    nc.scalar.activation(
                        dst, ps[:], mybir.ActivationFunctionType.Copy)
                else:
                    nc.vector.tensor_copy(dst, ps[:])
                for out_ap, src in stores:
                    nc.gpsimd.dma_start(out=out_ap, in_=src)

            def planes(cb, xt, lo, hi, qs):
                # plane0 everywhere; plane1 = Sign below qs, is_ge above
                nc.vector.tensor_scalar(
                    cb[:, 0, lo:hi], xt[:, lo:hi], T1, None,
                    mybir.AluOpType.is_ge)
                if lo < qs:
                    e = min(hi, qs)
                    nc.scalar.activation(
                        cb[:, 1, lo:e], xt[:, lo:e],
                        mybir.ActivationFunctionType.Sign,
                        bias=ACT_BIAS, scale=ACT_SCALE)
                if hi > qs:
                    s = max(lo, qs)
                    nc.vector.tensor_scalar(
                        cb[:, 1, s:hi], xt[:, s:hi], T2, None,
                        mybir.AluOpType.is_ge)

            pending = []
            for c, (cs0, width, qs) in enumerate(BLOCKS):
                halves = width // HALF
                last_blk = (c == n_blk - 1)
                xts = []
                for rb in range(RB):
                    rs = slice(rb * P, (rb + 1) * P)
                    xt = xp.tile([P, width], f32, name="xt")
                    # ALL loads on the sync ring: one HWDGE queue alone
                    # sustains ~404 GB/s; a second queue just splits the
                    # same 16 SDMA engines and scrambles completion order
                    nc.sync.dma_start(out=xt[:], in_=x[rs, cs0:cs0 + width])
                    xts.append(xt)
                cbs = []
                for rb in range(RB):
                    xt = xts[rb]
                    cb = cbp.tile([P, 2, width], fp8, name="cb")
                    if last_blk and rb >= RB - 2:
                        # tail tiles: per-half planes so the last psum
                        # group's matmuls start right after the last load
                        for h in range(halves):
                            planes(cb, xt, h * HALF, (h + 1) * HALF, qs)
                    else:
                        planes(cb, xt, 0, width, qs)
                    cbs.append(cb)
                sts = [stp.tile([len(grp) * OP, width], mybir.dt.uint8,
                                name="st")
                       for g, grp, _ in GROUPS]
                # group-major: PE executes its queue in strict FIFO order,
                # so the last group's (load-gated) matmuls must be emitted
                # AFTER everything that is ready earlier
                acts = _ACT_CASTS[width]
                for g, grp, row0 in GROUPS:
                    for h in range(halves):
                        while len(pending) > 3:
                            flush(pending.pop(0))
                        gp = len(grp) * OP
                        ps = psp.tile([gp, HALF], f32, name="ps")
                        for rl, rb in enumerate(grp):
                            pr = slice(rl * OP, (rl + 1) * OP)
                            for q in range(HALF // CH):
                                col = h * HALF + q * CH
                                pc = slice(q * CH, (q + 1) * CH)
                                nc.tensor.matmul(
                                    ps[pr, pc], wt[:],
                                    cbs[rb][:, 0, col:col + CH],
                                    start=True, stop=False)
                                nc.tensor.matmul(
                                    ps[pr, pc], wt[:],
                                    cbs[rb][:, 1, col:col + CH],
                                    start=False, stop=True)
                        on_act = (g * halves + h) in acts
                        stores = []
                        st = sts[g]
                        if last_blk and g == 2:
                            # critical-path stores: drain per half
                            stores.append((
                                o[row0:row0 + gp,
                                  cs0 + h * HALF:cs0 + (h + 1) * HALF],
                                st[:, h * HALF:(h + 1) * HALF]))
                        elif h == halves - 1:
                            stores.append((
                                o[row0:row0 + gp, cs0:cs0 + width], st[:]))
                        pending.append((ps, st, h, on_act, stores))
            while pending:
                flush(pending.pop(0))
    nc.compile()
    return nc


def _get_nc():
    if "nc" not in _BUILT:
        _BUILT["nc"] = build_nc()
    return _BUILT["nc"]


# code index offset per column: natural-code cols (beyond each block's
# sign coverage) use LUT6[3..5]; sign cols use {1,2,0} -> LUT6[0..2].
_NAT = np.zeros((1, COLS), dtype=np.uint8)
for _cs0, _w, _qs in BLOCKS:
    _NAT[0, _cs0 + _qs:_cs0 + _w] = 3


def _decode(packed: np.ndarray) -> np.ndarray:
    """[ROWS//4, COLS] u8 -> [ROWS, COLS] f32, bit-exact levels."""
    shifts = (2 * np.arange(PACK, dtype=np.uint8)).reshape(1, PACK, 1)
    codes = ((packed[:, None, :] >> shifts) & np.uint8(3))
    idx = codes + _NAT[:, None, :]
    return LUT6.take(idx).reshape(ROWS, COLS)


def kernel(inputs: np.ndarray, _trace: bool = False, _nc=None):
    assert inputs.shape == (ROWS, COLS) and inputs.dtype == np.float32
    nc = _nc if _nc is not None else _get_nc()
    wv = _weights()
    bv = np.full((P, 1), np.float32(ACT_BIAS), dtype=np.float32)
    in_maps = [
        {"inputs": np.ascontiguousarray(
            inputs[i * SHARD_ROWS:(i + 1) * SHARD_ROWS]),
         "w": wv, "bias": bv}
        for i in range(N_CORES)
    ]
    res = run_bass_kernel_spmd(nc, in_maps, list(range(N_CORES)), trace=_trace)
    packed = np.concatenate(
        [np.asarray(res.results[i]["out"]) for i in range(N_CORES)], axis=0)
    out = _decode(packed)
    if _trace:
        return out, res
    return out
